# revision 20
# baseline (speedup 1.0000x reference)
"""Trainium2 Bass kernel for nn_EquivariantMPNNEmbedding.

Strategy: pure data parallel over the walker (batch) dimension - 128 walkers
split as 16 per NeuronCore across 8 cores. All graph structure (edge lists)
is known on the host at call time, so gathers/scatters are compiled into
one-hot matmuls on the tensor engine; spherical harmonics and the pair-level
feature gathers are precomputed on the host (pure functions of the inputs).

Per-core layouts:
  edge tensors   : feature-major [f, w*Ep + e] for the radial MLPs, and
                   edge-chunk-major [128e, (w, cols)] for message math
  node tensors   : node-major [n, (w, c)] as gather matmul rhs,
                   feature-major [c, (w, n)] for h1 / one_el matmuls
"""

import numpy as np
import ml_dtypes

import concourse.bass as bass
import concourse.mybir as mybir
from concourse import bacc
from concourse.tile import TileContext
from concourse.masks import make_identity

F32 = mybir.dt.float32
BF16 = mybir.dt.bfloat16
AF = mybir.ActivationFunctionType
OP = mybir.AluOpType
BF = ml_dtypes.bfloat16

N_CORES = 8
B = 128
BW = B // N_CORES          # walkers per core (16)
N_EL = 32
N_ION = 8
C = 64
H = 128
F_EL = 64
FEE = 32
N_ITER = 3
R_SHORT = 0.5
R_LONG = 5.0
SQ3 = float(np.sqrt(3.0))

EPAD = (512, 512, 256)     # padded edges: same, diff, ion
NJ = (4, 4, 2)

_PROGRAM_CACHE = {}


# ---------------------------------------------------------------- device ---


def _build_program():
    nc = bacc.Bacc("TRN2", target_bir_lowering=False, debug=False)

    def din(name, shape, dt=BF16):
        return nc.dram_tensor(name, shape, dt, kind="ExternalInput").ap()

    inp = {}
    inp["fee"] = din("fee", [64, BW * 512])
    inp["fio"] = din("fio", [32, BW * 256])
    for t, tag in enumerate(("s", "d", "i")):
        J = NJ[t]
        # y values replicated 8x in the last dim (c-broadcast via 8-runs)
        inp[f"y0x_{tag}"] = din(f"y0x_{tag}", [128, J * BW * 8])
        inp[f"y1x_{tag}"] = din(f"y1x_{tag}", [128, J * 3 * BW * 8])
    inp["gsm"] = din("gsm", [32, 512])
    inp["gdf"] = din("gdf", [32, 512])
    inp["gio"] = din("gio", [8, 256])
    inp["ssm"] = din("ssm", [128, 4 * 32])
    inp["sdf"] = din("sdf", [128, 4 * 32])
    inp["sio"] = din("sio", [128, 2 * 32])
    inp["fel"] = din("fel", [64, BW * 32])
    inp["fioT"] = din("fioT", [32, 8])

    SDIM = (F_EL, H, H)
    NW = (2 * C, 4 * C, 4 * C)
    for n in range(N_ITER):
        inp[f"w1blk_{n}"] = din(f"w1blk_{n}", [64, 128])
        inp[f"w2blk_{n}"] = din(f"w2blk_{n}", [128, 128])
        inp[f"b1sd_{n}"] = din(f"b1sd_{n}", [128, 1], F32)
        inp[f"b2sd_{n}"] = din(f"b2sd_{n}", [128, 1], F32)
        inp[f"w3s_{n}"] = din(f"w3s_{n}", [64, NW[n]])
        inp[f"w3d_{n}"] = din(f"w3d_{n}", [128, NW[n]])
        for tag in ("s", "d"):
            inp[f"b3x{tag}_{n}"] = din(f"b3x{tag}_{n}", [128, NW[n]])
        inp[f"w1i_{n}"] = din(f"w1i_{n}", [32, 64])
        inp[f"b1i_{n}"] = din(f"b1i_{n}", [64, 1], F32)
        inp[f"w2i_{n}"] = din(f"w2i_{n}", [64, 64])
        inp[f"b2i_{n}"] = din(f"b2i_{n}", [64, 1], F32)
        inp[f"w3i_{n}"] = din(f"w3i_{n}", [64, 2 * C])
        inp[f"b3xi_{n}"] = din(f"b3xi_{n}", [128, 2 * C])
        inp[f"hws_{n}"] = din(f"hws_{n}", [SDIM[n], 64])
        if n >= 1:
            inp[f"hwv_{n}"] = din(f"hwv_{n}", [128, 64])
        inp[f"wio_{n}"] = din(f"wio_{n}", [32, 64])
        ins_dim = SDIM[n] + 3 * C
        ks = [128] * (ins_dim // 128) + ([ins_dim % 128] if ins_dim % 128 else [])
        for k, kk in enumerate(ks):
            inp[f"oes_{n}_{k}"] = din(f"oes_{n}_{k}", [kk, 128])
        if n < N_ITER - 1:
            inv_dim = (SDIM[n] if n >= 1 else 0) + 3 * C
            kv = [128] * (inv_dim // 128) + ([inv_dim % 128] if inv_dim % 128 else [])
            for k, kk in enumerate(kv):
                inp[f"oev_{n}_{k}"] = din(f"oev_{n}_{k}", [kk, 128])

    out = nc.dram_tensor("out", [BW * 32, H], F32, kind="ExternalOutput").ap()

    with TileContext(nc) as tc:
        _emit(nc, tc, inp, out)
    nc.compile()
    return nc


def _v4(ap):
    """[128, (w, 64)] -> [128, w, 8, 8]"""
    return ap.rearrange("p (w a b) -> p w a b", w=BW, a=8)


def _emit(nc, tc, inp, out):
    import contextlib

    ctx = contextlib.ExitStack()
    with ctx:
        P = ctx.enter_context(tc.tile_pool(name="persist", bufs=1))
        WRK = ctx.enter_context(tc.tile_pool(name="work", bufs=2))
        PS = ctx.enter_context(tc.tile_pool(name="ps", bufs=3, space="PSUM"))
        PS1 = ctx.enter_context(tc.tile_pool(name="ps1", bufs=3, space="PSUM"))
        PS2 = ctx.enter_context(tc.tile_pool(name="ps2", bufs=1, space="PSUM"))

        # ---- load persistent inputs to SBUF
        sb = {}
        for name, ap in inp.items():
            t = P.tile(list(ap.shape), ap.dtype, tag=f"in_{name}", name=f"in_{name}")
            nc.sync.dma_start(out=t[:], in_=ap[:])
            sb[name] = t

        ident = P.tile([128, 32], BF16)
        for q in range(4):
            make_identity(nc, ident[q * 32 : q * 32 + 32, :])
        identf = P.tile([128, 128], F32)
        make_identity(nc, identf[:])

        # ---- persistent working tiles
        A2 = P.tile([128, BW * 512], BF16)   # rows 0:64 same/ion, 64:128 diff

        sgb2 = [P.tile([128, BW * 64], BF16, tag=f"sgb{i}", name=f"sgb{i}") for i in range(2)]
        vgb2 = [P.tile([128, 3 * BW * 64], BF16, tag=f"vgb{i}", name=f"vgb{i}") for i in range(2)]
        wsb = P.tile([128, BW * 256], BF16)
        msg2 = [P.tile([128, BW * 256], BF16, tag=f"msg{i}", name=f"msg{i}") for i in range(2)]

        tw1 = P.tile([128, BW * 64], BF16)
        tQ = P.tile([128, BW * 64], BF16)
        tT = P.tile([128, BW * 64], BF16)
        tdot = P.tile([128, BW * 64], BF16)
        tP2 = P.tile([128, BW * 64], BF16)
        tmpva = P.tile([128, BW * 64], BF16)
        tmpvb = P.tile([128, BW * 64], BF16)
        tmv1, tmv2 = tmpva, tmpvb  # disjoint lifetimes: dot vs mv products

        h1n = P.tile([32, BW * 64], BF16)       # h1s node-major [n, (w,c)]
        vn = P.tile([32, 3 * BW * 64], BF16)    # h1v node-major [n, (d,w,c)]
        ionb = P.tile([8, BW * 64], BF16)       # ion embeds broadcast over w

        osb = [P.tile([96, BW * 64], BF16, tag=f"osb{t}", name=f"osb{t}") for t in range(3)]
        osb2 = [P.tile([32, BW * 64], BF16, tag=f"o2{t}", name=f"o2{t}") for t in range(3)]

        s_f32 = [P.tile([128, BW * 32], F32, tag=f"sf{i}", name=f"sf{i}") for i in range(2)]
        s_bf = P.tile([128, BW * 32], BF16)
        v_f32 = [P.tile([128, BW * 96], F32, tag=f"vf{i}", name=f"vf{i}") for i in range(2)]
        v_bf = P.tile([128, BW * 96], BF16)
        v_bfd = P.tile([128, BW * 96], BF16)   # (d, w, n)-major copy

        tA = P.tile([128, BW * 32], BF16)       # iter0 cat_s chunk0 [fel; ms]
        tBc = P.tile([128, BW * 32], BF16)      # cat_s chunk [ms|md] (iter0: [md|mi])
        tC = P.tile([64, BW * 32], BF16)        # cat_s chunk mi
        vA = P.tile([128, BW * 96], BF16)       # cat_v [msv | mdv]
        vB = P.tile([64, BW * 96], BF16)        # cat_v miv

        v_pre = P.tile([128, BW * 96], F32)     # pre-nonlinearity nv
        t_vscr = P.tile([128, BW * 96], F32)    # scratch: squares, scaled nv
        t_n2 = P.tile([128, BW * 32], F32)
        t_nrm = P.tile([128, BW * 32], F32)
        t_gn = P.tile([128, BW * 32], F32)
        t_rin = P.tile([128, BW * 32], F32)
        t_ratio = P.tile([128, BW * 32], F32)
        t_gs = P.tile([128, BW * 32], F32)
        t_eps = P.tile([128, 1], F32)
        nc.vector.memset(t_eps[:], 1e-12)

        TYPES = [
            dict(tag="s", E=512, J=4, g="gsm", s="ssm", nsrc=32),
            dict(tag="d", E=512, J=4, g="gdf", s="sdf", nsrc=32),
            dict(tag="i", E=256, J=2, g="gio", s="sio", nsrc=8),
        ]
        SDIM = (F_EL, H, H)
        NW = (2 * C, 4 * C, 4 * C)

        for n in range(N_ITER):
            sdim = SDIM[n]
            has_v = n >= 1

            # ---------------- phase A: node embeddings ----------------
            sfm = sb["fel"] if n == 0 else s_bf  # [sdim, (w, n)]
            for g in range(2):
                ph = PS.tile([64, 256], F32, tag="t2", name="ph")
                for pi in range(4):
                    pr = g * 4 + pi
                    nc.tensor.matmul(
                        ph[:, pi * 64 : pi * 64 + 64],
                        sfm[0:sdim, pr * 64 : pr * 64 + 64],
                        sb[f"hws_{n}"][:],
                        start=True,
                        stop=True,
                    )
                # rows 0:32 even walker, 32:64 odd walker of each pair
                p3 = ph[:].rearrange("p (r c) -> p r c", r=4)
                h3 = h1n[0:32, g * 512 : g * 512 + 512].rearrange(
                    "p (r c) -> p r c", r=8
                )
                nc.vector.tensor_copy(h3[:, 0::2, :], p3[0:32])
                nc.vector.tensor_copy(h3[:, 1::2, :], p3[32:64])
            if has_v:
                # repack v_bf (w,n,d) -> v_bfd (d,w,n): contiguous mini lhsT
                nc.vector.tensor_copy(
                    v_bfd[:].rearrange("p (d w x) -> p d w x", d=3, w=BW),
                    v_bf[:].rearrange("p (w x d) -> p d w x", w=BW, x=32),
                )
                for bidx in range(6):  # 6 batches of 4 pair-matmuls
                    pv = PS.tile([64, 256], F32, tag="t2", name="pv")
                    for k in range(4):
                        pr = bidx * 4 + k
                        nc.tensor.matmul(
                            pv[:, k * 64 : k * 64 + 64],
                            v_bfd[:, pr * 64 : pr * 64 + 64],
                            sb[f"hwv_{n}"][:],
                            start=True,
                            stop=True,
                        )
                    p3 = pv[:].rearrange("p (r c) -> p r c", r=4)
                    v3o = vn[0:32, bidx * 512 : bidx * 512 + 512].rearrange(
                        "p (r c) -> p r c", r=8
                    )
                    nc.vector.tensor_copy(v3o[:, 0::2, :], p3[0:32])
                    nc.vector.tensor_copy(v3o[:, 1::2, :], p3[32:64])
            pio = PS.tile([8, 64], F32, tag="t2", name="pio")
            nc.tensor.matmul(pio[:], sb["fioT"][:], sb[f"wio_{n}"][:], start=True, stop=True)
            tio = WRK.tile([8, 64], BF16, tag="tio")
            nc.scalar.activation(tio[:], pio[:], AF.Copy)
            nc.vector.tensor_copy(
                ionb[:].rearrange("p (w c) -> p w c", w=BW),
                tio[:].unsqueeze(1).broadcast_to((8, BW, 64)),
            )

            # ---------------- per edge-type ----------------
            for t, T in enumerate(TYPES):
                tag, E, J, nsrc = T["tag"], T["E"], T["J"], T["nsrc"]
                R = BW * E
                is_ion = tag == "i"
                light = is_ion or not has_v
                # -------- phase B: radial MLP (feature-major) --------
                if is_ion:
                    nw = 2 * C
                    w3 = sb[f"w3i_{n}"]
                    b3x = sb[f"b3xi_{n}"]
                    w1 = sb[f"w1i_{n}"][:]
                    b1, w2, b2 = sb[f"b1i_{n}"], sb[f"w2i_{n}"], sb[f"b2i_{n}"]
                    for ch in range(R // 512):
                        cs = slice(ch * 512, ch * 512 + 512)
                        p1 = PS.tile([64, 512], F32, tag="t2", name="p1")
                        nc.tensor.matmul(
                            p1[:], w1, sb["fio"][0:32, cs], start=True, stop=True
                        )
                        a1 = WRK.tile([64, 512], BF16, tag="a1")
                        nc.scalar.activation(a1[:], p1[:], AF.Silu, bias=b1[:])
                        p2 = PS.tile([64, 512], F32, tag="t2", name="p2")
                        nc.tensor.matmul(p2[:], w2[:], a1[:], start=True, stop=True)
                        nc.scalar.activation(A2[0:64, cs], p2[:], AF.Silu, bias=b2[:])
                else:
                    nw = NW[n]
                    w3 = sb[f"w3{tag}_{n}"]
                    b3x = sb[f"b3x{tag}_{n}"]
                    if tag == "s":
                        # paired same+diff MLP via block-diagonal weights
                        for ch in range(R // 512):
                            cs = slice(ch * 512, ch * 512 + 512)
                            p1 = PS.tile([128, 512], F32, tag="t2", name="p1")
                            nc.tensor.matmul(
                                p1[:], sb[f"w1blk_{n}"][:], sb["fee"][:, cs],
                                start=True, stop=True,
                            )
                            a1 = WRK.tile([128, 512], BF16, tag="a1")
                            nc.scalar.activation(
                                a1[:], p1[:], AF.Silu, bias=sb[f"b1sd_{n}"][:]
                            )
                            p2 = PS.tile([128, 512], F32, tag="t2", name="p2")
                            nc.tensor.matmul(
                                p2[:], sb[f"w2blk_{n}"][:], a1[:], start=True, stop=True
                            )
                            nc.scalar.activation(
                                A2[:, cs], p2[:], AF.Silu, bias=sb[f"b2sd_{n}"][:]
                            )

                # -------- phase C: per chunk j: gather, L3, messages, scatter
                gmat = sb[T["g"]]
                y0x = sb[f"y0x_{tag}"]
                y1x = sb[f"y1x_{tag}"]
                smat = sb[T["s"]]
                po = PS2.tile([128, BW * 64], F32, tag="po", name="po")
                for j in range(J):
                    sgb = sgb2[j % 2]
                    vgb = vgb2[j % 2]
                    msg = msg2[j % 2]
                    # gathers
                    for h in range(2):
                        psg = PS.tile([128, 512], F32, tag="t2", name="psg")
                        nc.tensor.matmul(
                            psg[:],
                            gmat[0:nsrc, j * 128 : j * 128 + 128],
                            (ionb if is_ion else h1n)[0:nsrc, h * 512 : h * 512 + 512],
                            start=True,
                            stop=True,
                        )
                        nc.vector.tensor_copy(sgb[:, h * 512 : h * 512 + 512], psg[:])
                    if not light:
                        for d in range(3):
                            for h in range(2):
                                pvg = PS.tile([128, 512], F32, tag="t2", name="pvg")
                                nc.tensor.matmul(
                                    pvg[:],
                                    gmat[0:nsrc, j * 128 : j * 128 + 128],
                                    vn[0:nsrc, d * 1024 + h * 512 : d * 1024 + h * 512 + 512],
                                    start=True,
                                    stop=True,
                                )
                                nc.vector.tensor_copy(
                                    vgb[:, d * 1024 + h * 512 : d * 1024 + h * 512 + 512],
                                    pvg[:],
                                )
                    # L3 -> wsb [128, (w, nw)]
                    a2r = 64 if tag == "d" else 0
                    w3mm = w3[64:128, :] if tag == "d" else w3[:]
                    for g in range(8):
                        pw = PS1.tile([128, 2 * 256], F32, tag="t4", name="pw")
                        for wi in range(2):
                            w = g * 2 + wi
                            nc.tensor.matmul(
                                pw[:, wi * nw : wi * nw + nw],
                                A2[a2r : a2r + 64, w * E + j * 128 : w * E + j * 128 + 128],
                                w3mm,
                                start=True,
                                stop=True,
                            )
                        nc.vector.tensor_tensor(
                            wsb[:, g * 2 * nw : g * 2 * nw + 2 * nw].rearrange(
                                "p (x c) -> p x c", x=2
                            ),
                            pw[:, : 2 * nw].rearrange("p (x c) -> p x c", x=2),
                            b3x[:].unsqueeze(1).broadcast_to((128, 2, nw)),
                            OP.add,
                        )
                    # products
                    wv = wsb[:, 0 : BW * nw].rearrange("p (w c) -> p w c", w=BW)
                    y0b = (
                        y0x[:, j * 128 : j * 128 + 128]
                        .rearrange("p (w b) -> p w b", w=BW)
                        .unsqueeze(2)
                        .broadcast_to((128, BW, 8, 8))
                    )
                    y1j = y1x[:, j * 384 : j * 384 + 384].rearrange(
                        "p (d w b) -> p d w b", d=3, w=BW
                    )
                    sgv = sgb[:].rearrange("p (w c) -> p w c", w=BW)
                    msgj = msg[:].rearrange("p (w c) -> p w c", w=BW)
                    # w~1 = w1 * Y0 ; Q = w2 * sg
                    nc.vector.tensor_tensor(
                        _v4(tw1[:]),
                        wv[:, :, 0:64].rearrange("p w (a b) -> p w a b", a=8),
                        y0b, OP.mult,
                    )
                    nc.vector.tensor_tensor(
                        tQ[:].rearrange("p (w c) -> p w c", w=BW),
                        wv[:, :, 64:128], sgv, OP.mult,
                    )
                    if not light:
                        nc.vector.tensor_tensor(
                            _v4(tT[:]),
                            wv[:, :, 192:256].rearrange("p w (a b) -> p w a b", a=8),
                            y0b, OP.mult,
                        )
                        # dot = sum_d vg_d * y1_d
                        for d in range(2):
                            y1b = (
                                y1j[:, d].unsqueeze(2).broadcast_to((128, BW, 8, 8))
                            )
                            nc.vector.tensor_tensor(
                                _v4((tmpva if d == 0 else tmpvb)[:]),
                                _v4(vgb[:, d * 1024 : d * 1024 + 1024]),
                                y1b, OP.mult,
                            )
                        nc.gpsimd.tensor_add(tdot[:], tmpva[:], tmpvb[:])
                        y1b = y1j[:, 2].unsqueeze(2).broadcast_to((128, BW, 8, 8))
                        nc.vector.tensor_tensor(
                            _v4(tmpva[:]),
                            _v4(vgb[:, 2048:3072]),
                            y1b, OP.mult,
                        )
                        nc.gpsimd.tensor_add(tdot[:], tdot[:], tmpva[:])
                        # ms = w~1*sg + w3*dot
                        nc.vector.tensor_tensor(
                            tmpvb[:].rearrange("p (w c) -> p w c", w=BW),
                            tw1[:].rearrange("p (w c) -> p w c", w=BW),
                            sgv, OP.mult,
                        )
                        nc.vector.tensor_tensor(
                            tP2[:].rearrange("p (w c) -> p w c", w=BW),
                            wv[:, :, 128:192],
                            tdot[:].rearrange("p (w c) -> p w c", w=BW),
                            OP.mult,
                        )
                        nc.gpsimd.tensor_add(
                            msgj[:, :, 0:64],
                            tmpvb[:].rearrange("p (w c) -> p w c", w=BW),
                            tP2[:].rearrange("p (w c) -> p w c", w=BW),
                        )
                        # mv_d = Q * Y1_d + T * vg_d
                        for d in range(3):
                            y1b = (
                                y1j[:, d].unsqueeze(2).broadcast_to((128, BW, 8, 8))
                            )
                            nc.vector.tensor_tensor(
                                _v4(tmv1[:]), _v4(tQ[:]), y1b, OP.mult,
                            )
                            nc.vector.tensor_mul(
                                tmv2[:].rearrange("p (w c) -> p w c", w=BW),
                                tT[:].rearrange("p (w c) -> p w c", w=BW),
                                vgb[:, d * 1024 : d * 1024 + 1024].rearrange(
                                    "p (w c) -> p w c", w=BW
                                ),
                            )
                            nc.gpsimd.tensor_add(
                                msgj[:, :, 64 + 64 * d : 128 + 64 * d],
                                tmv1[:].rearrange("p (w c) -> p w c", w=BW),
                                tmv2[:].rearrange("p (w c) -> p w c", w=BW),
                            )
                    else:
                        nc.vector.tensor_tensor(
                            msgj[:, :, 0:64],
                            tw1[:].rearrange("p (w c) -> p w c", w=BW),
                            sgv, OP.mult,
                        )
                        for d in range(3):
                            y1b = (
                                y1j[:, d].unsqueeze(2).broadcast_to((128, BW, 8, 8))
                            )
                            nc.vector.tensor_tensor(
                                msgj[:, :, 64 + 64 * d : 128 + 64 * d].rearrange(
                                    "p w (a b) -> p w a b", a=8
                                ),
                                _v4(tQ[:]), y1b, OP.mult,
                            )
                    # scatter j
                    st = smat[:, j * 32 : j * 32 + 32]
                    for h in range(2):
                        hw = slice(h * 8, h * 8 + 8)
                        hc = slice(h * 512, h * 512 + 512)
                        nc.tensor.matmul(
                            po[0:32, hc], st, msgj[:, hw, 0:64],
                            start=(j == 0), stop=(j == J - 1),
                            skip_group_check=True,
                        )
                        for d in range(3):
                            nc.tensor.matmul(
                                po[32 * (d + 1) : 32 * (d + 2), hc],
                                st,
                                msgj[:, hw, 64 + 64 * d : 128 + 64 * d],
                                start=(j == 0),
                                stop=(j == J - 1),
                                tile_position=(0, 32 * (d + 1)),
                                skip_group_check=True,
                            )
                nc.scalar.activation(osb[t][:], po[0:96, :], AF.Copy)
                nc.scalar.activation(osb2[t][:], po[96:128, :], AF.Copy)

            # ---------------- phase D: transposes into cat tiles ----------------
            if n == 0:
                nc.vector.tensor_copy(tA[0:64, :], sb["fel"][:])
                s_targets = [(tA, 64), (tBc, 0), (tBc, 64)]
            else:
                s_targets = [(tBc, 0), (tBc, 64), (tC, 0)]
            v_targets = [(vA, 0), (vA, 64), (vB, 0)]
            for t in range(3):
                dst, roff = s_targets[t]
                # 2-walker-packed transposes: in [32, 128] -> psum [128, 32]
                ptr = PS.tile([128, 256], BF16, tag="t2", name="ptr")
                for pr in range(8):
                    nc.tensor.transpose(
                        ptr[:, pr * 32 : pr * 32 + 32],
                        osb[t][0:32, pr * 128 : pr * 128 + 128],
                        ident[0:32, :],
                    )
                # rows 0:64 = even walkers' channels, 64:128 = odd walkers'
                pv3 = ptr[:].rearrange("p (r x) -> p r x", r=8)
                d3 = dst[roff : roff + 64, :].rearrange("p (w x) -> p w x", w=BW)
                nc.scalar.activation(d3[:, 0::2, :], pv3[0:64], AF.Copy)
                nc.scalar.activation(d3[:, 1::2, :], pv3[64:128], AF.Copy)
                dstv, voff = v_targets[t]
                dv = dstv[:].rearrange("p (w x d) -> p w x d", w=BW, x=32)
                for d in range(3):
                    if d < 2:
                        vsrc, vrow = osb[t], 32 + 32 * d
                    else:
                        vsrc, vrow = osb2[t], 0
                    ptv = PS.tile([128, 256], BF16, tag="t2", name="ptv")
                    for pr in range(8):
                        nc.tensor.transpose(
                            ptv[:, pr * 32 : pr * 32 + 32],
                            vsrc[vrow : vrow + 32, pr * 128 : pr * 128 + 128],
                            ident[vrow : vrow + 32, :],
                        )
                    pw3 = ptv[:].rearrange("p (r x) -> p r x", r=8)
                    nc.scalar.activation(dv[voff : voff + 64, 0::2, :, d], pw3[0:64], AF.Copy)
                    nc.scalar.activation(dv[voff : voff + 64, 1::2, :, d], pw3[64:128], AF.Copy)

            # ---------------- phase E: one_el + nonlinearity ----------------
            cur, nxt = s_f32[n % 2], s_f32[(n + 1) % 2]
            vcur, vnxt = v_f32[n % 2], v_f32[(n + 1) % 2]
            schunks = [tA, tBc] if n == 0 else [s_bf, tBc, tC]
            poe = PS1.tile([128, BW * 32], F32, tag="t4", name="poe")
            for k, chk in enumerate(schunks):
                kdim = sb[f"oes_{n}_{k}"].shape[0]
                nc.tensor.matmul(
                    poe[:],
                    sb[f"oes_{n}_{k}"][:],
                    chk[0:kdim, :],
                    start=(k == 0),
                    stop=(k == len(schunks) - 1),
                )
            if n == 1:
                nc.scalar.activation(t_gs[:], poe[:], AF.Gelu_apprx_tanh)
                nc.vector.tensor_add(nxt[:], t_gs[:], cur[:])
            else:
                nc.scalar.activation(nxt[:], poe[:], AF.Gelu_apprx_tanh)
            nc.scalar.activation(s_bf[:], nxt[:], AF.Copy)

            if n < N_ITER - 1:
                vchunks = [vA, vB] if n == 0 else [v_bf, vA, vB]
                for third in range(3):
                    ts_ = slice(third * 512, third * 512 + 512)
                    pov = PS1.tile([128, 512], F32, tag="t4", name="pov")
                    for k, chk in enumerate(vchunks):
                        kdim = sb[f"oev_{n}_{k}"].shape[0]
                        nc.tensor.matmul(
                            pov[:],
                            sb[f"oev_{n}_{k}"][:],
                            chk[0:kdim, ts_],
                            start=(k == 0),
                            stop=(k == len(vchunks) - 1),
                        )
                    nc.scalar.activation(v_pre[:, ts_], pov[:], AF.Copy)
                # gelu-norm: v *= gelu(|v|)/|v|
                nc.scalar.activation(t_vscr[:], v_pre[:], AF.Square)
                nc.vector.tensor_reduce(
                    t_n2[:],
                    t_vscr[:].rearrange("p (x d) -> p x d", d=3),
                    mybir.AxisListType.X,
                    OP.add,
                )
                nc.scalar.activation(t_nrm[:], t_n2[:], AF.Sqrt, bias=t_eps[:])
                nc.scalar.activation(t_gn[:], t_nrm[:], AF.Gelu_apprx_tanh)
                nc.vector.reciprocal(t_rin[:], t_nrm[:])
                nc.vector.tensor_mul(t_ratio[:], t_gn[:], t_rin[:])
                rb = t_ratio[:].unsqueeze(2).broadcast_to((128, BW * 32, 3))
                if n == 0:
                    nc.vector.tensor_tensor(
                        vnxt[:].rearrange("p (x d) -> p x d", d=3),
                        v_pre[:].rearrange("p (x d) -> p x d", d=3),
                        rb, OP.mult,
                    )
                else:
                    nc.vector.tensor_tensor(
                        t_vscr[:].rearrange("p (x d) -> p x d", d=3),
                        v_pre[:].rearrange("p (x d) -> p x d", d=3),
                        rb, OP.mult,
                    )
                    nc.vector.tensor_add(vnxt[:], t_vscr[:], vcur[:])
                nc.scalar.activation(v_bf[:], vnxt[:], AF.Copy)

        # ---------------- output: transpose [128H, (w,n)] -> [(w,n), 128H]
        fin = s_f32[N_ITER % 2]
        for g in range(4):
            pt = PS.tile([32, 512], F32, tag="t2", name="pt")
            for wi in range(4):
                w = g * 4 + wi
                nc.tensor.matmul(
                    pt[:, wi * 128 : wi * 128 + 128],
                    fin[:, w * 32 : w * 32 + 32],
                    identf[:],
                    is_transpose=True,
                )
            so = WRK.tile([32, 512], F32, tag="so")
            nc.scalar.activation(so[:], pt[:], AF.Copy)
            nc.sync.dma_start(
                out=out[g * 128 : g * 128 + 128, :].rearrange("(w x) h -> x w h", w=4),
                in_=so[:].rearrange("p (w h) -> p w h", w=4),
            )


# ---------------------------------------------------------------- host ---


def _sh_np(diff, dist):
    inv = np.where(dist > 0, 1.0 / np.maximum(dist, 1e-12), 0.0)
    Y1 = SQ3 * diff * inv[..., None]
    f_long = np.cos(0.5 * np.pi * np.minimum(dist / R_LONG, 1.0))
    Y0 = f_long
    Y1 = Y1 * (np.tanh(dist / R_SHORT) * f_long)[..., None]
    return Y0.astype(np.float32), Y1.astype(np.float32)


def _prep_host(inputs):
    """Build all per-core input arrays. Returns list of dicts (one per core)."""
    g = {k: np.asarray(v) for k, v in inputs.items() if k != "params"}
    params = inputs["params"]

    Y0ee, Y1ee = _sh_np(g["diff_el_el"], g["dist_el_el"])
    Y0ei, Y1ei = _sh_np(g["diff_el_ion"], g["dist_el_ion"])

    idx = [
        (g["same_tgt"].astype(np.int64), g["same_src"].astype(np.int64), Y0ee, Y1ee,
         g["feat_el_el"], 512, 32),
        (g["diff_tgt"].astype(np.int64), g["diff_src"].astype(np.int64), Y0ee, Y1ee,
         g["feat_el_el"], 512, 32),
        (g["ion_tgt"].astype(np.int64), g["ion_src"].astype(np.int64), Y0ei, Y1ei,
         g["feat_el_ion"], 256, 8),
    ]
    Y0t, Y1t, Ft, Gt, St = [], [], [], [], []
    for tgt, src, Y0g, Y1g, fg, Ep, nsrc in idx:
        E = len(tgt)
        y0 = np.zeros((B, Ep), np.float32)
        y1 = np.zeros((B, Ep, 3), np.float32)
        f = np.zeros((B, Ep, FEE), np.float32)
        y0[:, :E] = Y0g[:, tgt, src]
        y1[:, :E] = Y1g[:, tgt, src]
        f[:, :E] = fg[:, tgt, src]
        G = np.zeros((nsrc, Ep), np.float32)
        G[src, np.arange(E)] = 1.0
        S = np.zeros((Ep, 32), np.float32)
        S[np.arange(E), tgt] = 1.0
        Y0t.append(y0); Y1t.append(y1); Ft.append(f); Gt.append(G); St.append(S)

    def bf16(x):
        return np.ascontiguousarray(x, np.float32).astype(BF)

    shared = {}
    shared["gsm"] = bf16(Gt[0])
    shared["gdf"] = bf16(Gt[1])
    shared["gio"] = bf16(Gt[2])
    for name, tt in (("ssm", 0), ("sdf", 1), ("sio", 2)):
        S = St[tt]
        J = S.shape[0] // 128
        shared[name] = bf16(
            S.reshape(J, 128, 32).transpose(1, 0, 2).reshape(128, J * 32)
        )
    shared["fioT"] = bf16(np.asarray(inputs["feat_ion"]).T)

    for n in range(N_ITER):
        w1blk = np.zeros((64, 128), np.float32)
        w2blk = np.zeros((128, 128), np.float32)
        b1sd = np.zeros((128, 1), np.float32)
        b2sd = np.zeros((128, 1), np.float32)
        for tag, pkey in (("s", "rad_same"), ("d", "rad_diff"), ("i", "rad_ion")):
            p = params[pkey][n]
            Ws, bs = [np.asarray(a) for a in p["Ws"]], [np.asarray(a) for a in p["bs"]]
            if tag == "i":
                shared[f"w1i_{n}"] = bf16(Ws[0])
                shared[f"b1i_{n}"] = np.ascontiguousarray(bs[0][:, None], np.float32)
                shared[f"w2i_{n}"] = bf16(Ws[1])
                shared[f"b2i_{n}"] = np.ascontiguousarray(bs[1][:, None], np.float32)
            else:
                r0 = 0 if tag == "s" else 32
                c0 = 0 if tag == "s" else 64
                w1blk[r0 : r0 + 32, c0 : c0 + 64] = Ws[0]
                w2blk[c0 : c0 + 64, c0 : c0 + 64] = Ws[1]
                b1sd[c0 : c0 + 64, 0] = bs[0]
                b2sd[c0 : c0 + 64, 0] = bs[1]
            if tag == "d":
                w3p = np.zeros((128, Ws[2].shape[1]), np.float32)
                w3p[64:128] = Ws[2]
                shared[f"w3{tag}_{n}"] = bf16(w3p)
            else:
                shared[f"w3{tag}_{n}"] = bf16(Ws[2])
            shared[f"b3x{tag}_{n}"] = bf16(
                np.broadcast_to(bs[2][None, :], (128, bs[2].shape[0]))
            )
        shared[f"w1blk_{n}"] = bf16(w1blk)
        shared[f"w2blk_{n}"] = bf16(w2blk)
        shared[f"b1sd_{n}"] = b1sd
        shared[f"b2sd_{n}"] = b2sd
        shared[f"hws_{n}"] = bf16(np.asarray(params["h1"][n]["Ws"]))
        if n >= 1:
            shared[f"hwv_{n}"] = bf16(np.asarray(params["h1"][n]["Wv"]))
        shared[f"wio_{n}"] = bf16(np.asarray(params["ion"][n]))
        Woe = np.asarray(params["one_el"][n]["Ws"])
        off = 0
        k = 0
        while off < Woe.shape[0]:
            kk = min(128, Woe.shape[0] - off)
            shared[f"oes_{n}_{k}"] = bf16(Woe[off : off + kk])
            off += kk
            k += 1
        Wv = params["one_el"][n]["Wv"]
        if Wv is not None:
            Wv = np.asarray(Wv)
            off = 0
            k = 0
            while off < Wv.shape[0]:
                kk = min(128, Wv.shape[0] - off)
                shared[f"oev_{n}_{k}"] = bf16(Wv[off : off + kk])
                off += kk
                k += 1

    feat_el = np.asarray(inputs["feat_el"])
    in_maps = []
    for core in range(N_CORES):
        ws = slice(core * BW, (core + 1) * BW)
        m = dict(shared)
        fee = np.zeros((64, BW * 512), np.float32)
        fee[0:32] = Ft[0][ws].transpose(2, 0, 1).reshape(32, -1)
        fee[32:64] = Ft[1][ws].transpose(2, 0, 1).reshape(32, -1)
        m["fee"] = bf16(fee)
        m["fio"] = bf16(Ft[2][ws].transpose(2, 0, 1).reshape(32, -1))
        m["fel"] = bf16(feat_el[ws].transpose(2, 0, 1).reshape(64, -1))
        for tt, mt in enumerate(("s", "d", "i")):
            J = NJ[tt]
            y0 = Y0t[tt][ws]  # [BW, Ep]
            y1 = Y1t[tt][ws]  # [BW, Ep, 3]
            y0x = np.broadcast_to(
                y0.reshape(BW, J, 128).transpose(2, 1, 0)[:, :, :, None],
                (128, J, BW, 8),
            ).reshape(128, -1)
            m[f"y0x_{mt}"] = bf16(y0x)
            y1x = np.broadcast_to(
                y1.reshape(BW, J, 128, 3).transpose(2, 1, 3, 0)[..., None],
                (128, J, 3, BW, 8),
            ).reshape(128, -1)
            m[f"y1x_{mt}"] = bf16(y1x)
        in_maps.append(m)
    return in_maps


def kernel(**inputs):
    from concourse.bass_utils import run_bass_kernel_spmd

    if "prog" not in _PROGRAM_CACHE:
        _PROGRAM_CACHE["prog"] = _build_program()
    nc = _PROGRAM_CACHE["prog"]
    in_maps = _prep_host(inputs)
    res = run_bass_kernel_spmd(nc, in_maps, list(range(N_CORES)))
    outs = []
    for core in range(N_CORES):
        o = res.results[core]["out"].reshape(BW, 32, H)
        outs.append(o)
    return np.concatenate(outs, axis=0).astype(np.float32)


# revision 21
# speedup vs baseline: 1.2263x; 1.2263x over previous
"""Trainium2 Bass kernel for nn_EquivariantMPNNEmbedding.

Strategy: pure data parallel over the walker (batch) dimension - 128 walkers
split as 16 per NeuronCore across 8 cores. All graph structure (edge lists)
is known on the host at call time, so gathers/scatters are compiled into
one-hot matmuls on the tensor engine; spherical harmonics and the pair-level
feature gathers are precomputed on the host (pure functions of the inputs).

Per-core layouts:
  edge tensors   : feature-major [f, w*Ep + e] for the radial MLPs, and
                   edge-chunk-major [128e, (w, cols)] for message math
  node tensors   : node-major [n, (w, c)] as gather matmul rhs,
                   feature-major [c, (w, n)] for h1 / one_el matmuls
"""

import numpy as np
import ml_dtypes

import concourse.bass as bass
import concourse.mybir as mybir
from concourse import bacc
from concourse.tile import TileContext
from concourse.masks import make_identity

F32 = mybir.dt.float32
BF16 = mybir.dt.bfloat16
AF = mybir.ActivationFunctionType
OP = mybir.AluOpType
BF = ml_dtypes.bfloat16

N_CORES = 8
B = 128
BW = B // N_CORES          # walkers per core (16)
N_EL = 32
N_ION = 8
C = 64
H = 128
F_EL = 64
FEE = 32
N_ITER = 3
R_SHORT = 0.5
R_LONG = 5.0
SQ3 = float(np.sqrt(3.0))

EPAD = (512, 512, 256)     # padded edges: same, diff, ion
NJ = (4, 4, 2)

_PROGRAM_CACHE = {}


# ---------------------------------------------------------------- device ---


def _build_program():
    nc = bacc.Bacc("TRN2", target_bir_lowering=False, debug=False)

    def din(name, shape, dt=BF16):
        return nc.dram_tensor(name, shape, dt, kind="ExternalInput").ap()

    inp = {}
    inp["fee"] = din("fee", [64, BW * 512])
    inp["fio"] = din("fio", [32, BW * 256])
    for t, tag in enumerate(("s", "d", "i")):
        J = NJ[t]
        # y values replicated 8x in the last dim (c-broadcast via 8-runs)
        inp[f"y0x_{tag}"] = din(f"y0x_{tag}", [128, J * BW * 8])
        inp[f"y1x_{tag}"] = din(f"y1x_{tag}", [128, J * 3 * BW * 8])
    inp["gsm"] = din("gsm", [32, 512])
    inp["gdf"] = din("gdf", [32, 512])
    inp["gio"] = din("gio", [8, 256])
    inp["ssm"] = din("ssm", [128, 4 * 32])
    inp["sdf"] = din("sdf", [128, 4 * 32])
    inp["sio"] = din("sio", [128, 2 * 32])
    inp["fel"] = din("fel", [64, BW * 32])
    inp["fioT"] = din("fioT", [32, 8])

    SDIM = (F_EL, H, H)
    NW = (2 * C, 4 * C, 4 * C)
    for n in range(N_ITER):
        inp[f"w1blk_{n}"] = din(f"w1blk_{n}", [64, 128])
        inp[f"w2blk_{n}"] = din(f"w2blk_{n}", [128, 128])
        inp[f"b1sd_{n}"] = din(f"b1sd_{n}", [128, 1], F32)
        inp[f"b2sd_{n}"] = din(f"b2sd_{n}", [128, 1], F32)
        inp[f"w3s_{n}"] = din(f"w3s_{n}", [65, NW[n]])
        inp[f"w3d_{n}"] = din(f"w3d_{n}", [65, NW[n]])
        inp[f"w1i_{n}"] = din(f"w1i_{n}", [32, 64])
        inp[f"b1i_{n}"] = din(f"b1i_{n}", [64, 1], F32)
        inp[f"w2i_{n}"] = din(f"w2i_{n}", [64, 64])
        inp[f"b2i_{n}"] = din(f"b2i_{n}", [64, 1], F32)
        inp[f"w3i_{n}"] = din(f"w3i_{n}", [65, 2 * C])
        inp[f"hws_{n}"] = din(f"hws_{n}", [SDIM[n], 64])
        if n >= 1:
            inp[f"hwv_{n}"] = din(f"hwv_{n}", [128, 64])
        inp[f"wio_{n}"] = din(f"wio_{n}", [32, 64])
        ins_dim = SDIM[n] + 3 * C
        ks = [128] * (ins_dim // 128) + ([ins_dim % 128] if ins_dim % 128 else [])
        for k, kk in enumerate(ks):
            inp[f"oes_{n}_{k}"] = din(f"oes_{n}_{k}", [kk, 128])
        if n < N_ITER - 1:
            inv_dim = (SDIM[n] if n >= 1 else 0) + 3 * C
            kv = [128] * (inv_dim // 128) + ([inv_dim % 128] if inv_dim % 128 else [])
            for k, kk in enumerate(kv):
                inp[f"oev_{n}_{k}"] = din(f"oev_{n}_{k}", [kk, 128])

    out = nc.dram_tensor("out", [BW * 32, H], F32, kind="ExternalOutput").ap()

    with TileContext(nc) as tc:
        _emit(nc, tc, inp, out)
    nc.compile()
    return nc


def _v4(ap):
    """[128, (w, 64)] -> [128, w, 8, 8]"""
    return ap.rearrange("p (w a b) -> p w a b", w=BW, a=8)


def _emit(nc, tc, inp, out):
    import contextlib

    ctx = contextlib.ExitStack()
    with ctx:
        P = ctx.enter_context(tc.tile_pool(name="persist", bufs=1))
        WRK = ctx.enter_context(tc.tile_pool(name="work", bufs=2))
        PS = ctx.enter_context(tc.tile_pool(name="ps", bufs=3, space="PSUM"))
        PS1 = ctx.enter_context(tc.tile_pool(name="ps1", bufs=3, space="PSUM"))
        PS2 = ctx.enter_context(tc.tile_pool(name="ps2", bufs=1, space="PSUM"))

        # ---- load persistent inputs to SBUF
        sb = {}
        for name, ap in inp.items():
            t = P.tile(list(ap.shape), ap.dtype, tag=f"in_{name}", name=f"in_{name}")
            nc.sync.dma_start(out=t[:], in_=ap[:])
            sb[name] = t

        ident = P.tile([128, 32], BF16)
        for q in range(4):
            make_identity(nc, ident[q * 32 : q * 32 + 32, :])
        identf = P.tile([128, 128], F32)
        make_identity(nc, identf[:])

        # ---- persistent working tiles
        A2 = P.tile([65, BW * 512], BF16)       # same / ion
        A2d = P.tile([65, BW * 512], BF16)      # diff
        nc.vector.memset(A2[64:65, :], 1.0)
        nc.vector.memset(A2d[64:65, :], 1.0)

        sgb2 = [P.tile([128, BW * 64], BF16, tag=f"sgb{i}", name=f"sgb{i}") for i in range(2)]
        vgb = P.tile([128, 3 * BW * 64], BF16)
        wsb = P.tile([128, BW * 256], BF16)
        msg = P.tile([128, BW * 256], BF16)

        tw1 = P.tile([128, BW * 64], BF16)
        tQ = P.tile([128, BW * 64], BF16)
        tT = P.tile([128, BW * 64], BF16)
        tdot = P.tile([128, BW * 64], BF16)
        tP2 = P.tile([128, BW * 64], BF16)
        tmpva = P.tile([128, BW * 64], BF16)
        tmpvb = P.tile([128, BW * 64], BF16)
        tmv1, tmv2 = tmpva, tmpvb  # disjoint lifetimes: dot vs mv products

        h1n = P.tile([32, BW * 64], BF16)       # h1s node-major [n, (w,c)]
        vn = P.tile([32, 3 * BW * 64], BF16)    # h1v node-major [n, (d,w,c)]
        ionb = P.tile([8, BW * 64], BF16)       # ion embeds broadcast over w

        osb = [P.tile([96, BW * 64], BF16, tag=f"osb{t}", name=f"osb{t}") for t in range(3)]
        osb2 = [P.tile([32, BW * 64], BF16, tag=f"o2{t}", name=f"o2{t}") for t in range(3)]

        s_f32 = [P.tile([128, BW * 32], F32, tag=f"sf{i}", name=f"sf{i}") for i in range(2)]
        s_bf = P.tile([128, BW * 32], BF16)
        v_f32 = [P.tile([128, BW * 96], F32, tag=f"vf{i}", name=f"vf{i}") for i in range(2)]
        v_bf = P.tile([128, BW * 96], BF16)
        v_bfd = P.tile([128, BW * 96], BF16)   # (d, w, n)-major copy

        tA = P.tile([128, BW * 32], BF16)       # iter0 cat_s chunk0 [fel; ms]
        tBc = P.tile([128, BW * 32], BF16)      # cat_s chunk [ms|md] (iter0: [md|mi])
        tC = P.tile([64, BW * 32], BF16)        # cat_s chunk mi
        vA = P.tile([128, BW * 96], BF16)       # cat_v [msv | mdv]
        vB = P.tile([64, BW * 96], BF16)        # cat_v miv

        v_pre = P.tile([128, BW * 96], F32)     # pre-nonlinearity nv
        t_vscr = P.tile([128, BW * 96], F32)    # scratch: squares, scaled nv
        t_n2 = P.tile([128, BW * 32], F32)
        t_nrm = P.tile([128, BW * 32], F32)
        t_gn = P.tile([128, BW * 32], F32)
        t_rin = P.tile([128, BW * 32], F32)
        t_ratio = P.tile([128, BW * 32], F32)
        t_gs = P.tile([128, BW * 32], F32)
        t_eps = P.tile([128, 1], F32)
        nc.vector.memset(t_eps[:], 1e-12)

        TYPES = [
            dict(tag="s", E=512, J=4, g="gsm", s="ssm", nsrc=32),
            dict(tag="d", E=512, J=4, g="gdf", s="sdf", nsrc=32),
            dict(tag="i", E=256, J=2, g="gio", s="sio", nsrc=8),
        ]
        SDIM = (F_EL, H, H)
        NW = (2 * C, 4 * C, 4 * C)

        for n in range(N_ITER):
            sdim = SDIM[n]
            has_v = n >= 1

            # ---------------- phase A: node embeddings ----------------
            sfm = sb["fel"] if n == 0 else s_bf  # [sdim, (w, n)]
            for g in range(2):
                ph = PS.tile([64, 256], F32, tag="t2", name="ph")
                for pi in range(4):
                    pr = g * 4 + pi
                    nc.tensor.matmul(
                        ph[:, pi * 64 : pi * 64 + 64],
                        sfm[0:sdim, pr * 64 : pr * 64 + 64],
                        sb[f"hws_{n}"][:],
                        start=True,
                        stop=True,
                    )
                # rows 0:32 even walker, 32:64 odd walker of each pair
                p3 = ph[:].rearrange("p (r c) -> p r c", r=4)
                h3 = h1n[0:32, g * 512 : g * 512 + 512].rearrange(
                    "p (r c) -> p r c", r=8
                )
                nc.vector.tensor_copy(h3[:, 0::2, :], p3[0:32])
                nc.vector.tensor_copy(h3[:, 1::2, :], p3[32:64])
            if has_v:
                # repack v_bf (w,n,d) -> v_bfd (d,w,n): contiguous mini lhsT
                nc.vector.tensor_copy(
                    v_bfd[:].rearrange("p (d w x) -> p d w x", d=3, w=BW),
                    v_bf[:].rearrange("p (w x d) -> p d w x", w=BW, x=32),
                )
                for bidx in range(6):  # 6 batches of 4 pair-matmuls
                    pv = PS.tile([64, 256], F32, tag="t2", name="pv")
                    for k in range(4):
                        pr = bidx * 4 + k
                        nc.tensor.matmul(
                            pv[:, k * 64 : k * 64 + 64],
                            v_bfd[:, pr * 64 : pr * 64 + 64],
                            sb[f"hwv_{n}"][:],
                            start=True,
                            stop=True,
                        )
                    p3 = pv[:].rearrange("p (r c) -> p r c", r=4)
                    v3o = vn[0:32, bidx * 512 : bidx * 512 + 512].rearrange(
                        "p (r c) -> p r c", r=8
                    )
                    nc.vector.tensor_copy(v3o[:, 0::2, :], p3[0:32])
                    nc.vector.tensor_copy(v3o[:, 1::2, :], p3[32:64])
            pio = PS.tile([8, 64], F32, tag="t2", name="pio")
            nc.tensor.matmul(pio[:], sb["fioT"][:], sb[f"wio_{n}"][:], start=True, stop=True)
            tio = WRK.tile([8, 64], BF16, tag="tio")
            nc.scalar.activation(tio[:], pio[:], AF.Copy)
            nc.vector.tensor_copy(
                ionb[:].rearrange("p (w c) -> p w c", w=BW),
                tio[:].unsqueeze(1).broadcast_to((8, BW, 64)),
            )

            # ---------------- per edge-type ----------------
            for t, T in enumerate(TYPES):
                tag, E, J, nsrc = T["tag"], T["E"], T["J"], T["nsrc"]
                R = BW * E
                is_ion = tag == "i"
                light = is_ion or not has_v
                # -------- phase B: radial MLP (feature-major) --------
                if is_ion:
                    nw = 2 * C
                    w3 = sb[f"w3i_{n}"]
                    w1 = sb[f"w1i_{n}"][:]
                    b1, w2, b2 = sb[f"b1i_{n}"], sb[f"w2i_{n}"], sb[f"b2i_{n}"]
                    for ch in range(R // 512):
                        cs = slice(ch * 512, ch * 512 + 512)
                        p1 = PS.tile([64, 512], F32, tag="t2", name="p1")
                        nc.tensor.matmul(
                            p1[:], w1, sb["fio"][0:32, cs], start=True, stop=True
                        )
                        a1 = WRK.tile([64, 512], BF16, tag="a1")
                        nc.scalar.activation(a1[:], p1[:], AF.Silu, bias=b1[:])
                        p2 = PS.tile([64, 512], F32, tag="t2", name="p2")
                        nc.tensor.matmul(p2[:], w2[:], a1[:], start=True, stop=True)
                        nc.scalar.activation(A2[0:64, cs], p2[:], AF.Silu, bias=b2[:])
                else:
                    nw = NW[n]
                    w3 = sb[f"w3{tag}_{n}"]
                    if tag == "s":
                        # paired same+diff MLP via block-diagonal weights
                        for ch in range(R // 512):
                            cs = slice(ch * 512, ch * 512 + 512)
                            p1 = PS.tile([128, 512], F32, tag="t2", name="p1")
                            nc.tensor.matmul(
                                p1[:], sb[f"w1blk_{n}"][:], sb["fee"][:, cs],
                                start=True, stop=True,
                            )
                            a1 = WRK.tile([128, 512], BF16, tag="a1")
                            nc.scalar.activation(
                                a1[:], p1[:], AF.Silu, bias=sb[f"b1sd_{n}"][:]
                            )
                            p2 = PS.tile([128, 512], F32, tag="t2", name="p2")
                            nc.tensor.matmul(
                                p2[:], sb[f"w2blk_{n}"][:], a1[:], start=True, stop=True
                            )
                            nc.scalar.activation(
                                A2[0:64, cs], p2[0:64], AF.Silu,
                                bias=sb[f"b2sd_{n}"][0:64],
                            )
                            nc.scalar.activation(
                                A2d[0:64, cs], p2[64:128], AF.Silu,
                                bias=sb[f"b2sd_{n}"][64:128],
                            )

                # -------- phase C: per chunk j: gather, L3, messages, scatter
                gmat = sb[T["g"]]
                y0x = sb[f"y0x_{tag}"]
                y1x = sb[f"y1x_{tag}"]
                smat = sb[T["s"]]
                po = PS2.tile([128, BW * 64], F32, tag="po", name="po")
                for j in range(J):
                    sgb = sgb2[j % 2]
                    # gathers
                    for h in range(2):
                        psg = PS.tile([128, 512], F32, tag="t2", name="psg")
                        nc.tensor.matmul(
                            psg[:],
                            gmat[0:nsrc, j * 128 : j * 128 + 128],
                            (ionb if is_ion else h1n)[0:nsrc, h * 512 : h * 512 + 512],
                            start=True,
                            stop=True,
                        )
                        nc.vector.tensor_copy(sgb[:, h * 512 : h * 512 + 512], psg[:])
                    if not light:
                        for d in range(3):
                            for h in range(2):
                                pvg = PS.tile([128, 512], F32, tag="t2", name="pvg")
                                nc.tensor.matmul(
                                    pvg[:],
                                    gmat[0:nsrc, j * 128 : j * 128 + 128],
                                    vn[0:nsrc, d * 1024 + h * 512 : d * 1024 + h * 512 + 512],
                                    start=True,
                                    stop=True,
                                )
                                nc.vector.tensor_copy(
                                    vgb[:, d * 1024 + h * 512 : d * 1024 + h * 512 + 512],
                                    pvg[:],
                                )
                    # L3 -> wsb [128, (w, nw)]
                    a2t = A2d if tag == "d" else A2
                    for g in range(8):
                        pw = PS1.tile([128, 2 * 256], F32, tag="t4", name="pw")
                        for wi in range(2):
                            w = g * 2 + wi
                            nc.tensor.matmul(
                                pw[:, wi * nw : wi * nw + nw],
                                a2t[0:65, w * E + j * 128 : w * E + j * 128 + 128],
                                w3[:],
                                start=True,
                                stop=True,
                            )
                        nc.scalar.activation(
                            wsb[:, g * 2 * nw : g * 2 * nw + 2 * nw],
                            pw[:, : 2 * nw],
                            AF.Copy,
                        )
                    # products
                    wv = wsb[:, 0 : BW * nw].rearrange("p (w c) -> p w c", w=BW)
                    y0b = (
                        y0x[:, j * 128 : j * 128 + 128]
                        .rearrange("p (w b) -> p w b", w=BW)
                        .unsqueeze(2)
                        .broadcast_to((128, BW, 8, 8))
                    )
                    y1j = y1x[:, j * 384 : j * 384 + 384].rearrange(
                        "p (d w b) -> p d w b", d=3, w=BW
                    )
                    sgv = sgb[:].rearrange("p (w c) -> p w c", w=BW)
                    msgj = msg[:].rearrange("p (w c) -> p w c", w=BW)
                    # w~1 = w1 * Y0 ; Q = w2 * sg
                    nc.vector.tensor_tensor(
                        _v4(tw1[:]),
                        wv[:, :, 0:64].rearrange("p w (a b) -> p w a b", a=8),
                        y0b, OP.mult,
                    )
                    nc.vector.tensor_tensor(
                        tQ[:].rearrange("p (w c) -> p w c", w=BW),
                        wv[:, :, 64:128], sgv, OP.mult,
                    )
                    if not light:
                        nc.vector.tensor_tensor(
                            _v4(tT[:]),
                            wv[:, :, 192:256].rearrange("p w (a b) -> p w a b", a=8),
                            y0b, OP.mult,
                        )
                        # dot = sum_d vg_d * y1_d
                        for d in range(2):
                            y1b = (
                                y1j[:, d].unsqueeze(2).broadcast_to((128, BW, 8, 8))
                            )
                            nc.vector.tensor_tensor(
                                _v4((tmpva if d == 0 else tmpvb)[:]),
                                _v4(vgb[:, d * 1024 : d * 1024 + 1024]),
                                y1b, OP.mult,
                            )
                        nc.vector.tensor_add(tdot[:], tmpva[:], tmpvb[:])
                        y1b = y1j[:, 2].unsqueeze(2).broadcast_to((128, BW, 8, 8))
                        nc.vector.tensor_tensor(
                            _v4(tmpva[:]),
                            _v4(vgb[:, 2048:3072]),
                            y1b, OP.mult,
                        )
                        nc.vector.tensor_add(tdot[:], tdot[:], tmpva[:])
                        # ms = w~1*sg + w3*dot
                        nc.vector.tensor_tensor(
                            tmpvb[:].rearrange("p (w c) -> p w c", w=BW),
                            tw1[:].rearrange("p (w c) -> p w c", w=BW),
                            sgv, OP.mult,
                        )
                        nc.vector.tensor_tensor(
                            tP2[:].rearrange("p (w c) -> p w c", w=BW),
                            wv[:, :, 128:192],
                            tdot[:].rearrange("p (w c) -> p w c", w=BW),
                            OP.mult,
                        )
                        nc.vector.tensor_add(
                            msgj[:, :, 0:64],
                            tmpvb[:].rearrange("p (w c) -> p w c", w=BW),
                            tP2[:].rearrange("p (w c) -> p w c", w=BW),
                        )
                        # mv_d = Q * Y1_d + T * vg_d
                        for d in range(3):
                            y1b = (
                                y1j[:, d].unsqueeze(2).broadcast_to((128, BW, 8, 8))
                            )
                            nc.vector.tensor_tensor(
                                _v4(tmv1[:]), _v4(tQ[:]), y1b, OP.mult,
                            )
                            nc.vector.tensor_mul(
                                tmv2[:].rearrange("p (w c) -> p w c", w=BW),
                                tT[:].rearrange("p (w c) -> p w c", w=BW),
                                vgb[:, d * 1024 : d * 1024 + 1024].rearrange(
                                    "p (w c) -> p w c", w=BW
                                ),
                            )
                            nc.vector.tensor_add(
                                msgj[:, :, 64 + 64 * d : 128 + 64 * d],
                                tmv1[:].rearrange("p (w c) -> p w c", w=BW),
                                tmv2[:].rearrange("p (w c) -> p w c", w=BW),
                            )
                    else:
                        nc.vector.tensor_tensor(
                            msgj[:, :, 0:64],
                            tw1[:].rearrange("p (w c) -> p w c", w=BW),
                            sgv, OP.mult,
                        )
                        for d in range(3):
                            y1b = (
                                y1j[:, d].unsqueeze(2).broadcast_to((128, BW, 8, 8))
                            )
                            nc.vector.tensor_tensor(
                                msgj[:, :, 64 + 64 * d : 128 + 64 * d].rearrange(
                                    "p w (a b) -> p w a b", a=8
                                ),
                                _v4(tQ[:]), y1b, OP.mult,
                            )
                    # scatter j
                    st = smat[:, j * 32 : j * 32 + 32]
                    for h in range(2):
                        hw = slice(h * 8, h * 8 + 8)
                        hc = slice(h * 512, h * 512 + 512)
                        nc.tensor.matmul(
                            po[0:32, hc], st, msgj[:, hw, 0:64],
                            start=(j == 0), stop=(j == J - 1),
                            skip_group_check=True,
                        )
                        for d in range(3):
                            nc.tensor.matmul(
                                po[32 * (d + 1) : 32 * (d + 2), hc],
                                st,
                                msgj[:, hw, 64 + 64 * d : 128 + 64 * d],
                                start=(j == 0),
                                stop=(j == J - 1),
                                tile_position=(0, 32 * (d + 1)),
                                skip_group_check=True,
                            )
                nc.scalar.activation(osb[t][:], po[0:96, :], AF.Copy)
                nc.scalar.activation(osb2[t][:], po[96:128, :], AF.Copy)

            # ---------------- phase D: transposes into cat tiles ----------------
            if n == 0:
                nc.vector.tensor_copy(tA[0:64, :], sb["fel"][:])
                s_targets = [(tA, 64), (tBc, 0), (tBc, 64)]
            else:
                s_targets = [(tBc, 0), (tBc, 64), (tC, 0)]
            v_targets = [(vA, 0), (vA, 64), (vB, 0)]
            for t in range(3):
                dst, roff = s_targets[t]
                # 2-walker-packed transposes: in [32, 128] -> psum [128, 32]
                ptr = PS.tile([128, 256], BF16, tag="t2", name="ptr")
                for pr in range(8):
                    nc.tensor.transpose(
                        ptr[:, pr * 32 : pr * 32 + 32],
                        osb[t][0:32, pr * 128 : pr * 128 + 128],
                        ident[0:32, :],
                    )
                # rows 0:64 = even walkers' channels, 64:128 = odd walkers'
                pv3 = ptr[:].rearrange("p (r x) -> p r x", r=8)
                d3 = dst[roff : roff + 64, :].rearrange("p (w x) -> p w x", w=BW)
                nc.scalar.activation(d3[:, 0::2, :], pv3[0:64], AF.Copy)
                nc.scalar.activation(d3[:, 1::2, :], pv3[64:128], AF.Copy)
                dstv, voff = v_targets[t]
                dv = dstv[:].rearrange("p (w x d) -> p w x d", w=BW, x=32)
                for d in range(3):
                    if d < 2:
                        vsrc, vrow = osb[t], 32 + 32 * d
                    else:
                        vsrc, vrow = osb2[t], 0
                    ptv = PS.tile([128, 256], BF16, tag="t2", name="ptv")
                    for pr in range(8):
                        nc.tensor.transpose(
                            ptv[:, pr * 32 : pr * 32 + 32],
                            vsrc[vrow : vrow + 32, pr * 128 : pr * 128 + 128],
                            ident[vrow : vrow + 32, :],
                        )
                    pw3 = ptv[:].rearrange("p (r x) -> p r x", r=8)
                    nc.scalar.activation(dv[voff : voff + 64, 0::2, :, d], pw3[0:64], AF.Copy)
                    nc.scalar.activation(dv[voff : voff + 64, 1::2, :, d], pw3[64:128], AF.Copy)

            # ---------------- phase E: one_el + nonlinearity ----------------
            cur, nxt = s_f32[n % 2], s_f32[(n + 1) % 2]
            vcur, vnxt = v_f32[n % 2], v_f32[(n + 1) % 2]
            schunks = [tA, tBc] if n == 0 else [s_bf, tBc, tC]
            poe = PS1.tile([128, BW * 32], F32, tag="t4", name="poe")
            for k, chk in enumerate(schunks):
                kdim = sb[f"oes_{n}_{k}"].shape[0]
                nc.tensor.matmul(
                    poe[:],
                    sb[f"oes_{n}_{k}"][:],
                    chk[0:kdim, :],
                    start=(k == 0),
                    stop=(k == len(schunks) - 1),
                )
            if n == 1:
                nc.scalar.activation(t_gs[:], poe[:], AF.Gelu_apprx_tanh)
                nc.vector.tensor_add(nxt[:], t_gs[:], cur[:])
            else:
                nc.scalar.activation(nxt[:], poe[:], AF.Gelu_apprx_tanh)
            nc.scalar.activation(s_bf[:], nxt[:], AF.Copy)

            if n < N_ITER - 1:
                vchunks = [vA, vB] if n == 0 else [v_bf, vA, vB]
                for third in range(3):
                    ts_ = slice(third * 512, third * 512 + 512)
                    pov = PS1.tile([128, 512], F32, tag="t4", name="pov")
                    for k, chk in enumerate(vchunks):
                        kdim = sb[f"oev_{n}_{k}"].shape[0]
                        nc.tensor.matmul(
                            pov[:],
                            sb[f"oev_{n}_{k}"][:],
                            chk[0:kdim, ts_],
                            start=(k == 0),
                            stop=(k == len(vchunks) - 1),
                        )
                    nc.scalar.activation(v_pre[:, ts_], pov[:], AF.Copy)
                # gelu-norm: v *= gelu(|v|)/|v|
                nc.scalar.activation(t_vscr[:], v_pre[:], AF.Square)
                nc.vector.tensor_reduce(
                    t_n2[:],
                    t_vscr[:].rearrange("p (x d) -> p x d", d=3),
                    mybir.AxisListType.X,
                    OP.add,
                )
                nc.scalar.activation(t_nrm[:], t_n2[:], AF.Sqrt, bias=t_eps[:])
                nc.scalar.activation(t_gn[:], t_nrm[:], AF.Gelu_apprx_tanh)
                nc.vector.reciprocal(t_rin[:], t_nrm[:])
                nc.vector.tensor_mul(t_ratio[:], t_gn[:], t_rin[:])
                rb = t_ratio[:].unsqueeze(2).broadcast_to((128, BW * 32, 3))
                if n == 0:
                    nc.vector.tensor_tensor(
                        vnxt[:].rearrange("p (x d) -> p x d", d=3),
                        v_pre[:].rearrange("p (x d) -> p x d", d=3),
                        rb, OP.mult,
                    )
                else:
                    nc.vector.tensor_tensor(
                        t_vscr[:].rearrange("p (x d) -> p x d", d=3),
                        v_pre[:].rearrange("p (x d) -> p x d", d=3),
                        rb, OP.mult,
                    )
                    nc.vector.tensor_add(vnxt[:], t_vscr[:], vcur[:])
                nc.scalar.activation(v_bf[:], vnxt[:], AF.Copy)

        # ---------------- output: transpose [128H, (w,n)] -> [(w,n), 128H]
        fin = s_f32[N_ITER % 2]
        for g in range(4):
            pt = PS.tile([32, 512], F32, tag="t2", name="pt")
            for wi in range(4):
                w = g * 4 + wi
                nc.tensor.matmul(
                    pt[:, wi * 128 : wi * 128 + 128],
                    fin[:, w * 32 : w * 32 + 32],
                    identf[:],
                    is_transpose=True,
                )
            so = WRK.tile([32, 512], F32, tag="so")
            nc.scalar.activation(so[:], pt[:], AF.Copy)
            nc.sync.dma_start(
                out=out[g * 128 : g * 128 + 128, :].rearrange("(w x) h -> x w h", w=4),
                in_=so[:].rearrange("p (w h) -> p w h", w=4),
            )


# ---------------------------------------------------------------- host ---


def _sh_np(diff, dist):
    inv = np.where(dist > 0, 1.0 / np.maximum(dist, 1e-12), 0.0)
    Y1 = SQ3 * diff * inv[..., None]
    f_long = np.cos(0.5 * np.pi * np.minimum(dist / R_LONG, 1.0))
    Y0 = f_long
    Y1 = Y1 * (np.tanh(dist / R_SHORT) * f_long)[..., None]
    return Y0.astype(np.float32), Y1.astype(np.float32)


def _prep_host(inputs):
    """Build all per-core input arrays. Returns list of dicts (one per core)."""
    g = {k: np.asarray(v) for k, v in inputs.items() if k != "params"}
    params = inputs["params"]

    Y0ee, Y1ee = _sh_np(g["diff_el_el"], g["dist_el_el"])
    Y0ei, Y1ei = _sh_np(g["diff_el_ion"], g["dist_el_ion"])

    idx = [
        (g["same_tgt"].astype(np.int64), g["same_src"].astype(np.int64), Y0ee, Y1ee,
         g["feat_el_el"], 512, 32),
        (g["diff_tgt"].astype(np.int64), g["diff_src"].astype(np.int64), Y0ee, Y1ee,
         g["feat_el_el"], 512, 32),
        (g["ion_tgt"].astype(np.int64), g["ion_src"].astype(np.int64), Y0ei, Y1ei,
         g["feat_el_ion"], 256, 8),
    ]
    Y0t, Y1t, Ft, Gt, St = [], [], [], [], []
    for tgt, src, Y0g, Y1g, fg, Ep, nsrc in idx:
        E = len(tgt)
        y0 = np.zeros((B, Ep), np.float32)
        y1 = np.zeros((B, Ep, 3), np.float32)
        f = np.zeros((B, Ep, FEE), np.float32)
        y0[:, :E] = Y0g[:, tgt, src]
        y1[:, :E] = Y1g[:, tgt, src]
        f[:, :E] = fg[:, tgt, src]
        G = np.zeros((nsrc, Ep), np.float32)
        G[src, np.arange(E)] = 1.0
        S = np.zeros((Ep, 32), np.float32)
        S[np.arange(E), tgt] = 1.0
        Y0t.append(y0); Y1t.append(y1); Ft.append(f); Gt.append(G); St.append(S)

    def bf16(x):
        return np.ascontiguousarray(x, np.float32).astype(BF)

    shared = {}
    shared["gsm"] = bf16(Gt[0])
    shared["gdf"] = bf16(Gt[1])
    shared["gio"] = bf16(Gt[2])
    for name, tt in (("ssm", 0), ("sdf", 1), ("sio", 2)):
        S = St[tt]
        J = S.shape[0] // 128
        shared[name] = bf16(
            S.reshape(J, 128, 32).transpose(1, 0, 2).reshape(128, J * 32)
        )
    shared["fioT"] = bf16(np.asarray(inputs["feat_ion"]).T)

    for n in range(N_ITER):
        w1blk = np.zeros((64, 128), np.float32)
        w2blk = np.zeros((128, 128), np.float32)
        b1sd = np.zeros((128, 1), np.float32)
        b2sd = np.zeros((128, 1), np.float32)
        for tag, pkey in (("s", "rad_same"), ("d", "rad_diff"), ("i", "rad_ion")):
            p = params[pkey][n]
            Ws, bs = [np.asarray(a) for a in p["Ws"]], [np.asarray(a) for a in p["bs"]]
            if tag == "i":
                shared[f"w1i_{n}"] = bf16(Ws[0])
                shared[f"b1i_{n}"] = np.ascontiguousarray(bs[0][:, None], np.float32)
                shared[f"w2i_{n}"] = bf16(Ws[1])
                shared[f"b2i_{n}"] = np.ascontiguousarray(bs[1][:, None], np.float32)
            else:
                r0 = 0 if tag == "s" else 32
                c0 = 0 if tag == "s" else 64
                w1blk[r0 : r0 + 32, c0 : c0 + 64] = Ws[0]
                w2blk[c0 : c0 + 64, c0 : c0 + 64] = Ws[1]
                b1sd[c0 : c0 + 64, 0] = bs[0]
                b2sd[c0 : c0 + 64, 0] = bs[1]
            shared[f"w3{tag}_{n}"] = bf16(
                np.concatenate([Ws[2], bs[2][None, :]], axis=0)
            )
        shared[f"w1blk_{n}"] = bf16(w1blk)
        shared[f"w2blk_{n}"] = bf16(w2blk)
        shared[f"b1sd_{n}"] = b1sd
        shared[f"b2sd_{n}"] = b2sd
        shared[f"hws_{n}"] = bf16(np.asarray(params["h1"][n]["Ws"]))
        if n >= 1:
            shared[f"hwv_{n}"] = bf16(np.asarray(params["h1"][n]["Wv"]))
        shared[f"wio_{n}"] = bf16(np.asarray(params["ion"][n]))
        Woe = np.asarray(params["one_el"][n]["Ws"])
        off = 0
        k = 0
        while off < Woe.shape[0]:
            kk = min(128, Woe.shape[0] - off)
            shared[f"oes_{n}_{k}"] = bf16(Woe[off : off + kk])
            off += kk
            k += 1
        Wv = params["one_el"][n]["Wv"]
        if Wv is not None:
            Wv = np.asarray(Wv)
            off = 0
            k = 0
            while off < Wv.shape[0]:
                kk = min(128, Wv.shape[0] - off)
                shared[f"oev_{n}_{k}"] = bf16(Wv[off : off + kk])
                off += kk
                k += 1

    feat_el = np.asarray(inputs["feat_el"])
    in_maps = []
    for core in range(N_CORES):
        ws = slice(core * BW, (core + 1) * BW)
        m = dict(shared)
        fee = np.zeros((64, BW * 512), np.float32)
        fee[0:32] = Ft[0][ws].transpose(2, 0, 1).reshape(32, -1)
        fee[32:64] = Ft[1][ws].transpose(2, 0, 1).reshape(32, -1)
        m["fee"] = bf16(fee)
        m["fio"] = bf16(Ft[2][ws].transpose(2, 0, 1).reshape(32, -1))
        m["fel"] = bf16(feat_el[ws].transpose(2, 0, 1).reshape(64, -1))
        for tt, mt in enumerate(("s", "d", "i")):
            J = NJ[tt]
            y0 = Y0t[tt][ws]  # [BW, Ep]
            y1 = Y1t[tt][ws]  # [BW, Ep, 3]
            y0x = np.broadcast_to(
                y0.reshape(BW, J, 128).transpose(2, 1, 0)[:, :, :, None],
                (128, J, BW, 8),
            ).reshape(128, -1)
            m[f"y0x_{mt}"] = bf16(y0x)
            y1x = np.broadcast_to(
                y1.reshape(BW, J, 128, 3).transpose(2, 1, 3, 0)[..., None],
                (128, J, 3, BW, 8),
            ).reshape(128, -1)
            m[f"y1x_{mt}"] = bf16(y1x)
        in_maps.append(m)
    return in_maps


def kernel(**inputs):
    from concourse.bass_utils import run_bass_kernel_spmd

    if "prog" not in _PROGRAM_CACHE:
        _PROGRAM_CACHE["prog"] = _build_program()
    nc = _PROGRAM_CACHE["prog"]
    in_maps = _prep_host(inputs)
    res = run_bass_kernel_spmd(nc, in_maps, list(range(N_CORES)))
    outs = []
    for core in range(N_CORES):
        o = res.results[core]["out"].reshape(BW, 32, H)
        outs.append(o)
    return np.concatenate(outs, axis=0).astype(np.float32)
